# revision 1
# baseline (speedup 1.0000x reference)
"""Trainium2 Bass kernel for nn_Decoder_59760174957314 (gnn_message_passing).

Reference computation:
    hi = emb @ W1[:E]                 # [B, N, H]
    hj = emb @ W1[E:]                 # [B, N, H]
    h  = relu(hi[:, :, None] + hj[:, None, :] + b1)   # [B, N, N, H]
    out = sigmoid(h @ W2 + b2)[..., 0]                # [B, N, N]

Strategy (8 cores, data-parallel over (batch, i-half); each core computes a
[512, 1024] slab of the output):
  - The tiny GEMMs (hi/hj) are done host-side; the kernel's real work is the
    134M-element relu intermediate and its weighted reduction over H.
  - SBUF layout: partition p = (i32, h4) = 32 i-rows x 4 h-channels; free = j.
    Per (round r, col-group c, h-quartet q) a single DVE tensor_scalar
    (bf16, 4x mode) computes T = relu(hjb_rep + hi_scalar_per_partition).
    A minority of these run on the Scalar engine (activation Relu with
    per-partition bias) to offload the DVE.
  - TensorE reduces over h: matmul(psum[32c:32c+32] += wsel_q.T @ T) with
    PSUM accumulation over the 8 h-quartets; the four col-groups map to the
    four 32-column PE sub-array groups and stream concurrently.
  - ScalarE applies sigmoid straight out of PSUM; HWDGE DMA stores 256 KB
    tiles to HBM.
"""

import sys

if "/opt/trn_rl_repo" not in sys.path:
    sys.path.insert(0, "/opt/trn_rl_repo")

from contextlib import ExitStack

import ml_dtypes
import numpy as np

import bass_rust
import concourse.bass as bass
import concourse.mybir as mybir
import concourse.tile as tile
from concourse.bass_utils import run_bass_kernel_spmd

B, N, E, H = 4, 1024, 16, 32
NCORES = 8
ROWS = 512  # i-rows per core
NR = 4      # rounds of 128 i-rows
NCG = 4     # PE column groups (32 i-rows each)
NQ = 8      # h-quartets (4 h each)
JBLK = 512  # psum free size (one fp32 bank)

F32 = mybir.dt.float32
BF16 = mybir.dt.bfloat16
BF16_NP = ml_dtypes.bfloat16


def _engine_for(c: int, q: int) -> str:
    # Produce-work split tuned to engine rates (DVE ~401 G elem/s,
    # ScalarE ~114, GpSimd ~100): per round 22 units on DVE, 5 on ScalarE,
    # 5 on GpSimd, interleaved evenly along execution order t = q*4 + c.
    t = q * 4 + c
    if t in (2, 8, 14, 20, 29):
        return "P"
    if t in (5, 11, 17, 23):
        return "A"
    return "V"


# Packed input byte layout (per partition), loaded by exactly TWO DMAs —
# walrus caps sync-wait commands at one per instruction, so the input deps
# must collapse onto very few semaphores. Chunk 1 (small, lands fast) holds
# everything needed to start computing: hisc/wsel/b2 + the first two rep
# quartets; chunk 2 holds the rest of rep and overlaps with compute.
OFF_HISC = 0
OFF_WSEL = OFF_HISC + 128 * 4            # 512
OFF_B2 = OFF_WSEL + NQ * 32 * 2          # 1024
OFF_REP = OFF_B2 + 16                    # 1040 (pad keeps rep 16B-aligned)
TOTB = OFF_REP + NQ * N * 2              # 17424
CHUNK1 = OFF_REP + 2 * N * 2             # 5136 (incl. rep q0, q1)


def _build_nc():
    nc = bass.Bass("TRN2", debug=False)
    inp_d = nc.dram_tensor("inp", [128, TOTB], mybir.dt.uint8, kind="ExternalInput").ap()
    out_d = nc.dram_tensor("out", [ROWS, N], F32, kind="ExternalOutput").ap()

    with tile.TileContext(nc) as tc, ExitStack() as ctx:
        const = ctx.enter_context(tc.tile_pool(name="const", bufs=1))
        # bufs=9: the first slot reuse must come after the first chunk-2
        # consumer, so no tensor_scalar carries both a WAR wait and a DMA wait.
        tpool = ctx.enter_context(tc.tile_pool(name="tp", bufs=9))
        # bufs=20: ACT-produced tiles are never reused, so ScalarE relu ops
        # carry at most one sync wait (the ACT instruction struct budget is
        # the tightest in walrus codegen).
        apool = ctx.enter_context(tc.tile_pool(name="ap", bufs=20))
        gpool = ctx.enter_context(tc.tile_pool(name="gp", bufs=4))
        # bufs=4: one [128, 1024] two-bank psum tile per round (4 x 2 = all 8
        # banks, no reuse) — matmuls never wait on a sigmoid's read of a
        # recycled bank, and each round needs only ONE sigmoid (FD=1024).
        ppool = ctx.enter_context(tc.tile_pool(name="pp", bufs=4, space="PSUM"))

        # Sigmoid outputs land here; each (round, j-chunk) slice is stored by
        # its own DMA, all merged onto one completion semaphore post-pass.
        sgbuf = const.tile([128, NR * 2 * JBLK], F32, tag="sgbuf", name="sgbuf")

        inp_t = const.tile([128, TOTB], mybir.dt.uint8, tag="inp", name="inp_t")
        nc.sync.dma_start(inp_t[:, :CHUNK1], inp_d[:, :CHUNK1])
        nc.sync.dma_start(inp_t[:, CHUNK1:], inp_d[:, CHUNK1:])
        rep_t = inp_t[:, OFF_REP:TOTB].bitcast(BF16)          # [128, NQ*N]
        hisc_t = inp_t[:, OFF_HISC:OFF_WSEL].bitcast(F32)     # [128, 128]
        wsel_t = inp_t[:, OFF_WSEL:OFF_B2].bitcast(BF16)      # [128, NQ*32]
        b2_t = inp_t[:, OFF_B2 : OFF_B2 + 4].bitcast(F32)     # [128, 1]

        # ScalarE warm-up: make ACT's first instruction depend only on
        # chunk 1, so no later ACT relu ever needs two DMA waits.
        act_scratch = const.tile([128, 1], F32, tag="scr", name="act_scratch")
        nc.scalar.copy(act_scratch[:], b2_t[:])


        for r in range(NR):
            ps = ppool.tile([128, 2 * JBLK], F32, tag="ps", name=f"ps{r}")
            for q in range(NQ):
                for c in range(NCG):
                    tcol = r * 32 + c * 8 + q
                    eng = _engine_for(c, q)
                    pool = {"A": apool, "P": gpool, "V": tpool}[eng]
                    t_rel = pool.tile(
                        [128, N], BF16, tag="T" + eng,
                        name=f"T{r}_{q}_{c}",
                    )
                    src = rep_t[:, q * N : (q + 1) * N]
                    hi_col = hisc_t[:, tcol : tcol + 1]
                    if eng == "A":
                        nc.scalar.activation(
                            t_rel[:],
                            src,
                            mybir.ActivationFunctionType.Relu,
                            bias=hi_col,
                            scale=1.0,
                        )
                    else:
                        veng = nc.vector if eng == "V" else nc.gpsimd
                        veng.tensor_scalar(
                            t_rel[:],
                            src,
                            hi_col,
                            0.0,
                            mybir.AluOpType.add,
                            mybir.AluOpType.max,
                        )
                    for jc in range(2):
                        nc.tensor.matmul(
                            ps[c * 32 : (c + 1) * 32, jc * JBLK : (jc + 1) * JBLK],
                            wsel_t[:, q * 32 : (q + 1) * 32],
                            t_rel[:, jc * JBLK : (jc + 1) * JBLK],
                            start=(q == 0),
                            stop=(q == NQ - 1),
                            tile_position=(0, 32 * c),
                            # The sim's psum zero-region group check is bank
                            # granular and partition-base blind; HW accumulation
                            # is per-element (has_written), so partition-sliced
                            # concurrent groups are legal.
                            skip_group_check=True,
                        )
            nc.scalar.activation(
                sgbuf[:, r * N : (r + 1) * N],
                ps[:],
                mybir.ActivationFunctionType.Sigmoid,
                bias=b2_t[:, 0:1],
                scale=1.0,
            )
            nc.sync.dma_start(
                out_d[r * 128 : (r + 1) * 128, :],
                sgbuf[:, r * N : (r + 1) * N],
            )
    _strip_redundant_self_waits(nc)
    _merge_out_dma_sems(nc)
    return nc


# Engine semaphores incremented only by that engine's own (serial, in-order)
# instruction stream. A wait on one of these BY an instruction that itself
# updates the same semaphore is redundant: every increment counted by the
# threshold comes from a program-order predecessor on the same engine, which
# has necessarily completed. Tile emits these anyway (tile releases aggregate
# writer+reader sems without cross-proc transitivity), and walrus codegen
# only has ONE sync-wait slot per S3D3 instruction — so strip them.
# DMAHW lanes qualify too: all sync-engine DMAs share one HWDGE ring whose
# per-SDMA-engine descriptor FIFOs preserve increment order, so a threshold on
# a lane this DMA itself increments is already implied by ring order.
_ENGINE_SEM_PREFIXES = (
    "DVE_", "Activation_", "PE_", "Pool_", "SP_sequencer_", "DMAHW", "DMASW",
)


def _strip_redundant_self_waits(nc):
    for blk in nc.m.functions[0].blocks:
        for ins in blk.instructions:
            si = ins.sync_info
            if si is None or len(si.on_wait) <= 1:
                continue
            own = {u.ant_name for u in si.on_update}
            keep = [
                w
                for w in si.on_wait
                if not (
                    w.ant_name in own
                    and w.ant_name.startswith(_ENGINE_SEM_PREFIXES)
                )
            ]
            if len(keep) != len(si.on_wait):
                ins.sync_info = bass_rust.SyncInfo(
                    on_wait=keep, on_update=list(si.on_update)
                )

def _merge_out_dma_sems(nc):
    """Point every output DMA's completion increment at ONE semaphore lane,
    then rewrite the kernel-tail drain (which otherwise waits on every
    active semaphore — far over walrus's one-wait budget) to wait on that
    single lane at the total-increment threshold. Threshold counting needs
    no completion-order assumption: lane >= 16*n means all n DMAs' 16
    per-engine increments fired, i.e. every output byte landed. Everything
    else in the kernel is transitively upstream of the sigmoid->store
    chain, so no other wait is needed."""
    out_dmas = []
    for blk in nc.m.functions[0].blocks:
        for ins in blk.instructions:
            if type(ins).__name__ != "InstDMACopy":
                continue
            dest = ins.outs[0]
            name = getattr(dest, "memref", None) or getattr(
                getattr(dest, "tensor", None), "name", ""
            )
            if isinstance(name, str) and name.startswith("out"):
                out_dmas.append(ins)
    assert out_dmas, "no output DMAs found"
    canon = list(out_dmas[-1].sync_info.on_update)
    assert len(canon) == 1
    lane = canon[0].ant_name
    orig_lanes = set()
    for ins in out_dmas:
        upd = list(ins.sync_info.on_update)
        assert len(upd) == 1, ins.name
        orig_lanes.add(upd[0].ant_name)
        ins.sync_info = bass_rust.SyncInfo(
            on_wait=list(ins.sync_info.on_wait), on_update=list(canon)
        )
    # Only drains may observe the out-DMA lanes we just rewired.
    moved = orig_lanes - {lane}
    total = 0
    for blk in nc.m.functions[0].blocks:
        for ins in blk.instructions:
            si = ins.sync_info
            if si is None:
                continue
            for u in si.on_update:
                if u.ant_name == lane:
                    total += u.update_value
            if type(ins).__name__ != "InstDrain":
                for w in si.on_wait:
                    assert w.ant_name not in moved or w.wait_value <= 16, (
                        ins.name,
                        w.ant_name,
                    )
    final_wait = bass_rust.SyncWait(
        sync_type="semaphore",
        id=canon[0].id,
        ant_name=lane,
        wait_mode="sem-ge-imm",
        wait_value=total,
        wait_reg=None,
    )
    for blk in nc.m.functions[0].blocks:
        for ins in blk.instructions:
            if type(ins).__name__ != "InstDrain" or ins.sync_info is None:
                continue
            w = list(ins.sync_info.on_wait)
            if len(w) <= 1:
                continue
            ins.sync_info = bass_rust.SyncInfo(
                on_wait=[final_wait], on_update=list(ins.sync_info.on_update)
            )


_NC_CACHE = {}


def _get_nc():
    if "nc" not in _NC_CACHE:
        _NC_CACHE["nc"] = _build_nc()
    return _NC_CACHE["nc"]


def _prep_core(core, embeddings, W1, b1, W2, b2):
    b = core // 2
    i0 = (core % 2) * ROWS
    emb = np.asarray(embeddings[b], np.float32)          # [N, E]
    hi = emb @ np.asarray(W1[:E], np.float32)            # [N, H]
    hjb = emb @ np.asarray(W1[E:], np.float32) + np.asarray(b1, np.float32)

    hjbT = np.ascontiguousarray(hjb.T)                   # [H, N]
    rep = np.empty((128, NQ * N), np.float32)
    for q in range(NQ):
        rep[:, q * N : (q + 1) * N] = np.tile(hjbT[4 * q : 4 * q + 4, :], (32, 1))

    hisc = np.empty((128, 128), np.float32)
    for r in range(NR):
        for c in range(NCG):
            base = i0 + 128 * r + 32 * c
            for q in range(NQ):
                tcol = r * 32 + c * 8 + q
                hisc[:, tcol] = hi[base : base + 32, 4 * q : 4 * q + 4].reshape(-1)

    w2 = np.asarray(W2, np.float32).reshape(H)
    wsel = np.zeros((128, NQ * 32), np.float32)
    for q in range(NQ):
        for m in range(32):
            wsel[4 * m : 4 * m + 4, q * 32 + m] = w2[4 * q : 4 * q + 4]

    b2rep = np.full((128, 1), float(np.asarray(b2).reshape(-1)[0]), np.float32)

    blob = np.zeros((128, TOTB), np.uint8)
    blob[:, OFF_HISC:OFF_WSEL] = np.ascontiguousarray(hisc).view(np.uint8)
    blob[:, OFF_WSEL:OFF_B2] = wsel.astype(BF16_NP).view(np.uint8)
    blob[:, OFF_B2 : OFF_B2 + 4] = b2rep.view(np.uint8)
    blob[:, OFF_REP:TOTB] = rep.astype(BF16_NP).view(np.uint8)
    return {"inp": blob}


def kernel(embeddings, W1, b1, W2, b2):
    nc = _get_nc()
    in_maps = [
        _prep_core(core, embeddings, W1, b1, W2, b2) for core in range(NCORES)
    ]
    res = run_bass_kernel_spmd(nc, in_maps, list(range(NCORES)))
    out = np.empty((B, N, N), np.float32)
    for core in range(NCORES):
        b = core // 2
        i0 = (core % 2) * ROWS
        out[b, i0 : i0 + ROWS, :] = res.results[core]["out"]
    return out



# revision 16
# speedup vs baseline: 1.0221x; 1.0221x over previous
"""Trainium2 Bass kernel for nn_Decoder_59760174957314 (gnn_message_passing).

Reference computation:
    hi = emb @ W1[:E]                 # [B, N, H]
    hj = emb @ W1[E:]                 # [B, N, H]
    h  = relu(hi[:, :, None] + hj[:, None, :] + b1)   # [B, N, N, H]
    out = sigmoid(h @ W2 + b2)[..., 0]                # [B, N, N]

Strategy (8 cores, data-parallel over (batch, i-half); each core computes a
[512, 1024] slab of the output):
  - Host computes the tiny GEMMs (hi/hjb) and packs |W2|-scaled operands; the
    kernel computes T = |w_h|*relu(s) tiles on DVE/ScalarE/GpSimd and reduces
    over h on TensorE with per-slot signed selector matrices (sign(w) lives in
    the selector rows, so all elementwise units are a uniform add+max).
  - Channels are sign-sorted into 8 quartet slots; pairs of same-engine,
    sign-pure tiles are pre-folded by DVE tensor_tensor adds, halving the
    PE column count (PE serial time is the cost-model bottleneck).
  - A greedy planner balances DVE/ScalarE/GpSimd clocks and folds until PE
    time meets the elementwise makespan.
  - ScalarE applies sigmoid from PSUM; HWDGE DMA stores tiles to HBM.
"""

import sys

if "/opt/trn_rl_repo" not in sys.path:
    sys.path.insert(0, "/opt/trn_rl_repo")

from contextlib import ExitStack

import ml_dtypes
import numpy as np

import bass_rust
import concourse.bass as bass
import concourse.mybir as mybir
import concourse.tile as tile
from concourse.bass_utils import run_bass_kernel_spmd

B, N, E, H = 4, 1024, 16, 32
NCORES = 8
ROWS = 512   # i-rows per core
NR = 4       # rounds (psum tiles) of 128 i-rows
NBLK = 16    # i-blocks of 32 rows
NQ = 8       # channel quartet slots
JBLK = 512   # matmul moving-dim chunk

F32 = mybir.dt.float32
BF16 = mybir.dt.bfloat16
BF16_NP = ml_dtypes.bfloat16

# cost-model constants (ns) used by the build-time planner
_C_V, _C_A, _C_P, _C_FOLD, _C_PE = 327.0, 1038.0, 1517.0, 593.0, 426.7
_ACT_SIGMOID = 4000.0

# blob layout: TWO mirrored chunks, each self-contained (own hisc/sel/b2
# copy + 4 rep slots) so every consumer waits on exactly ONE DMA sem.
# chunk 0 goes out on the SP HWDGE queue, chunk 1 on Activation's — the
# two queues run in parallel, halving input-DMA latency.
OFF_HISC = 0
OFF_SEL = OFF_HISC + 128 * 4              # 512:  hisc fp32 [128, 128]
OFF_B2 = OFF_SEL + 10 * 32 * 2            # 1152: 10 selectors bf16 [128, 32]
OFF_REP = 1184                            # pad to 16B
CHK = OFF_REP + 4 * N * 2                 # 9376 bytes per chunk
TOTB = 2 * CHK
SEL_POS, SEL_NEG = 8, 9                   # selector ids for pure +/- folds


def _make_plan(w):
    """Channel permutation + per-(block,slot) engine assignment + folds."""
    w = np.asarray(w, np.float64).reshape(H)
    perm = np.argsort(-np.sign(w), kind="stable")  # pos channels first
    sgn = np.sign(w[perm]).astype(int)
    sgn[sgn == 0] = 1
    slot_class = []
    for q in range(NQ):
        s = sgn[4 * q : 4 * q + 4]
        slot_class.append(int(s[0]) if np.all(s == s[0]) else 0)

    best = None
    for reserve in range(0, 22000, 1500):
        tV, tA, tP = float(reserve), _ACT_SIGMOID, 0.0
        assign = {}
        for b in range(NBLK):
            for q in range(NQ):
                cand = [(tV + _C_V, "V"), (tA + _C_A, "A"), (tP + _C_P, "P")]
                t, e = min(cand)
                assign[(b, q)] = e
                if e == "V":
                    tV = t
                elif e == "A":
                    tA = t
                else:
                    tP = t
        tV -= reserve
        # fold chains per (block, class): seed with a same-producer pair
        # (one wait sem), extend with any same-class tile (DVE self-input
        # is stripped), each step -1 PE tile at +_C_FOLD DVE time.
        groups = {}
        for (b, q), e in assign.items():
            if slot_class[q] != 0:
                groups.setdefault((b, slot_class[q]), []).append((q, e))
        chains = {}  # (b, cls) -> [q, q, ...]
        pe = NBLK * NQ * _C_PE
        prog = True
        while prog and pe > max(tV + _C_FOLD, tA, tP):
            prog = False
            for key, tiles_left in groups.items():
                if pe <= max(tV + _C_FOLD, tA, tP):
                    break
                if key in chains:
                    if tiles_left:
                        chains[key].append(tiles_left.pop()[0])
                    else:
                        continue
                else:
                    # need a same-producer pair to seed
                    by_e = {}
                    seed = None
                    for q, e in tiles_left:
                        if e in by_e:
                            seed = (by_e[e], q)
                            break
                        by_e[e] = q
                    if seed is None:
                        continue
                    tiles_left[:] = [t for t in tiles_left if t[0] not in seed]
                    chains[key] = list(seed)
                tV += _C_FOLD
                pe -= _C_PE
                prog = True
        mk = max(tV, tA, tP, pe)
        if best is None or mk < best[0]:
            best = (mk, assign, [(b, c, qs) for (b, c), qs in chains.items()])
    _, assign, folds = best
    return perm, sgn, slot_class, assign, folds


def _build_nc(plan):
    slot_class, assign, folds = plan[2], plan[3], plan[4]
    fold_by_block = {}
    for b, c, qs in folds:
        fold_by_block.setdefault(b, []).append((c, list(qs)))

    # per-round tile counts decide pool sizes (all of a round's tiles stay
    # live until its matmuls run; +50% lets the next round's units start)
    nV = nA = nP = 0
    for r in range(NR):
        cnt = {"V": 0, "A": 0, "P": 0}
        for q in range(NQ):
            for c in range(4):
                cnt[assign[(r * 4 + c, q)]] += 1
        nV, nA, nP = max(nV, cnt["V"]), max(nA, cnt["A"]), max(nP, cnt["P"])
    nF = max(
        (
            sum(
                len(qs) - 1
                for _, qs in
                (x for c in range(4) for x in fold_by_block.get(r * 4 + c, []))
            )
            for r in range(NR)
        ),
        default=0,
    )

    nc = bass.Bass("TRN2", debug=False)
    inp_d = nc.dram_tensor("inp", [128, TOTB], mybir.dt.uint8, kind="ExternalInput").ap()
    out_d = nc.dram_tensor("out", [ROWS, N], F32, kind="ExternalOutput").ap()

    with tile.TileContext(nc) as tc, ExitStack() as ctx:
        const = ctx.enter_context(tc.tile_pool(name="const", bufs=1))
        vpool = ctx.enter_context(tc.tile_pool(name="vp", bufs=nV + 4))
        apool = ctx.enter_context(tc.tile_pool(name="ap", bufs=nA + 2))
        gpool = ctx.enter_context(tc.tile_pool(name="gp", bufs=nP + 2))
        fpool = ctx.enter_context(tc.tile_pool(name="fp", bufs=nF + 2))
        ppool = ctx.enter_context(tc.tile_pool(name="pp", bufs=4, space="PSUM"))

        sgbuf = const.tile([128, NR * N], F32, tag="sgbuf", name="sgbuf")
        inp_t = const.tile([128, TOTB], mybir.dt.uint8, tag="inp", name="inp_t")
        nc.sync.dma_start(inp_t[:, :CHK], inp_d[:, :CHK])
        nc.scalar.dma_start(inp_t[:, CHK:], inp_d[:, CHK:])
        hisc_t, sel_t, rep_t = [], [], []
        for k in range(2):
            base = k * CHK
            hisc_t.append(inp_t[:, base + OFF_HISC : base + OFF_SEL].bitcast(F32))
            sel_t.append(inp_t[:, base + OFF_SEL : base + OFF_B2].bitcast(BF16))
            rep_t.append(inp_t[:, base + OFF_REP : base + CHK].bitcast(BF16))
        b2_t = inp_t[:, OFF_B2 : OFF_B2 + 4].bitcast(F32)   # [128, 1]

        # ScalarE warm-up: first ACT instruction depends only on chunk 1.
        act_scratch = const.tile([128, 1], F32, tag="scr", name="act_scratch")
        nc.scalar.copy(act_scratch[:], b2_t[:])

        block_tiles = {}
        ps_tiles = {}

        def emit_units(blk):
            tiles = {}
            for q in range(NQ):
                eng = assign[(blk, q)]
                pool = {"V": vpool, "A": apool, "P": gpool}[eng]
                t_rel = pool.tile([128, N], BF16, tag="T" + eng,
                                  name=f"T{blk}_{q}")
                k = q // 4
                src = rep_t[k][:, (q % 4) * N : (q % 4 + 1) * N]
                hcol = blk * NQ + q
                hi_col = hisc_t[k][:, hcol : hcol + 1]
                if eng == "A":
                    nc.scalar.activation(
                        t_rel[:], src, mybir.ActivationFunctionType.Relu,
                        bias=hi_col, scale=1.0,
                    )
                else:
                    veng = nc.vector if eng == "V" else nc.gpsimd
                    veng.tensor_scalar(
                        t_rel[:], src, hi_col, 0.0,
                        mybir.AluOpType.add, mybir.AluOpType.max,
                    )
                tiles[q] = (t_rel, q)  # selector id = slot id
            block_tiles[blk] = tiles

        def emit_tail(blk):
            # folds (producers are a block ahead by now — no DVE stall),
            # then this block's matmuls into its round's psum tile.
            r, c = blk // 4, blk % 4
            tiles = block_tiles.pop(blk)
            for cls, qs in fold_by_block.get(blk, []):
                t1, _ = tiles.pop(qs[0])
                for step, q2 in enumerate(qs[1:]):
                    t2, _ = tiles.pop(q2)
                    tf = fpool.tile([128, N], BF16, tag="TF",
                                    name=f"F{blk}_{qs[0]}_{step}")
                    nc.vector.tensor_tensor(tf[:], t1[:], t2[:],
                                            mybir.AluOpType.add)
                    t1 = tf
                tiles[qs[0]] = (t1, SEL_POS if cls > 0 else SEL_NEG)
            if r not in ps_tiles:
                ps_tiles[r] = ppool.tile([128, 2 * JBLK], F32, tag="ps",
                                         name=f"ps{r}")
            ps = ps_tiles[r]
            keys = sorted(tiles)
            for ti, k in enumerate(keys):
                t_ap, sel_id = tiles[k]
                sel_chunk = sel_t[sel_id // 4] if sel_id < NQ else sel_t[0]
                sel = sel_chunk[:, sel_id * 32 : (sel_id + 1) * 32]
                for jc in range(2):
                    nc.tensor.matmul(
                        ps[c * 32 : (c + 1) * 32, jc * JBLK : (jc + 1) * JBLK],
                        sel,
                        t_ap[:, jc * JBLK : (jc + 1) * JBLK],
                        start=(ti == 0),
                        stop=(ti == len(keys) - 1),
                        tile_position=(0, 32 * c),
                        skip_group_check=True,
                    )
            if c == 3:
                nc.scalar.activation(
                    sgbuf[:, r * N : (r + 1) * N], ps[:],
                    mybir.ActivationFunctionType.Sigmoid,
                    bias=b2_t[:, 0:1], scale=1.0,
                )
                nc.sync.dma_start(
                    out_d[r * 128 : (r + 1) * 128, :],
                    sgbuf[:, r * N : (r + 1) * N],
                )

        for blk in range(NBLK):
            emit_units(blk)
            if blk >= 1:
                emit_tail(blk - 1)
        emit_tail(NBLK - 1)
    _strip_redundant_self_waits(nc)
    _merge_out_dma_sems(nc)
    return nc


_ENGINE_SEM_PREFIXES = (
    "DVE_", "Activation_", "PE_", "Pool_", "SP_sequencer_", "DMAHW", "DMASW",
)


def _strip_redundant_self_waits(nc):
    for blk in nc.m.functions[0].blocks:
        for ins in blk.instructions:
            si = ins.sync_info
            if si is None or len(si.on_wait) <= 1:
                continue
            own = {u.ant_name for u in si.on_update}
            keep = [
                w for w in si.on_wait
                if not (w.ant_name in own
                        and w.ant_name.startswith(_ENGINE_SEM_PREFIXES))
            ]
            if len(keep) != len(si.on_wait):
                ins.sync_info = bass_rust.SyncInfo(
                    on_wait=keep, on_update=list(si.on_update)
                )


def _merge_out_dma_sems(nc):
    """Collapse output-DMA completion sems onto one lane; rewrite the drain
    to a single threshold wait (walrus one-wait budget)."""
    out_dmas = []
    for blk in nc.m.functions[0].blocks:
        for ins in blk.instructions:
            if type(ins).__name__ != "InstDMACopy":
                continue
            dest = ins.outs[0]
            name = getattr(dest, "memref", None) or getattr(
                getattr(dest, "tensor", None), "name", ""
            )
            if isinstance(name, str) and name.startswith("out"):
                out_dmas.append(ins)
    assert out_dmas, "no output DMAs found"
    canon = list(out_dmas[-1].sync_info.on_update)
    assert len(canon) == 1
    lane = canon[0].ant_name
    for ins in out_dmas:
        ins.sync_info = bass_rust.SyncInfo(
            on_wait=list(ins.sync_info.on_wait), on_update=list(canon)
        )
    total = 0
    for blk in nc.m.functions[0].blocks:
        for ins in blk.instructions:
            si = ins.sync_info
            if si is None:
                continue
            for u in si.on_update:
                if u.ant_name == lane:
                    total += u.update_value
    final_wait = bass_rust.SyncWait(
        sync_type="semaphore", id=canon[0].id, ant_name=lane,
        wait_mode="sem-ge-imm", wait_value=total, wait_reg=None,
    )
    for blk in nc.m.functions[0].blocks:
        for ins in blk.instructions:
            if type(ins).__name__ != "InstDrain" or ins.sync_info is None:
                continue
            w = list(ins.sync_info.on_wait)
            if len(w) <= 1:
                continue
            ins.sync_info = bass_rust.SyncInfo(
                on_wait=[final_wait], on_update=list(ins.sync_info.on_update)
            )


_NC_CACHE = {}


def _get_nc(plan=None):
    if "nc" not in _NC_CACHE:
        assert plan is not None
        _NC_CACHE["nc"] = _build_nc(plan)
    return _NC_CACHE["nc"]


def _prep_core(core, embeddings, W1, b1, W2, b2, perm, sgn):
    b = core // 2
    i0 = (core % 2) * ROWS
    emb = np.asarray(embeddings[b], np.float32)          # [N, E]
    hi = emb @ np.asarray(W1[:E], np.float32)            # [N, H]
    hjb = emb @ np.asarray(W1[E:], np.float32) + np.asarray(b1, np.float32)
    w = np.asarray(W2, np.float32).reshape(H)
    aw = np.abs(w)[perm]                                 # |w| per slot-row

    # rep[p, q*N + j] = |w_ch| * hjb[j, ch],  ch = perm[4q + p%4]
    rep = np.empty((128, NQ * N), np.float32)
    for q in range(NQ):
        for rr in range(4):
            ch = perm[4 * q + rr]
            row = aw[4 * q + rr] * hjb[:, ch]
            rep[rr::4, q * N : (q + 1) * N] = row[None, :]

    # hisc[p=(4i+r), blk*NQ+q] = |w_ch| * hi[i_glob, ch]
    hisc = np.empty((128, 128), np.float32)
    for blk in range(NBLK):
        base = i0 + 32 * blk
        for q in range(NQ):
            col = np.empty(128, np.float32)
            for rr in range(4):
                ch = perm[4 * q + rr]
                col[rr::4] = aw[4 * q + rr] * hi[base : base + 32, ch]
            hisc[:, blk * NQ + q] = col

    # selectors: 8 per-slot + pure +/-.
    sel = np.zeros((128, 10 * 32), np.float32)
    for sid in range(10):
        for p in range(128):
            i, rr = p // 4, p % 4
            if sid < NQ:
                s = float(sgn[4 * sid + rr])
            else:
                s = 1.0 if sid == SEL_POS else -1.0
            sel[p, sid * 32 + i] = s

    b2rep = np.full((128, 1), float(np.asarray(b2).reshape(-1)[0]), np.float32)

    blob = np.zeros((128, TOTB), np.uint8)
    for k in range(2):
        base = k * CHK
        blob[:, base + OFF_HISC : base + OFF_SEL] = (
            np.ascontiguousarray(hisc).view(np.uint8)
        )
        blob[:, base + OFF_SEL : base + OFF_B2] = sel.astype(BF16_NP).view(np.uint8)
        blob[:, base + OFF_B2 : base + OFF_B2 + 4] = b2rep.view(np.uint8)
        blob[:, base + OFF_REP : base + CHK] = (
            rep[:, k * 4 * N : (k + 1) * 4 * N].astype(BF16_NP).view(np.uint8)
        )
    return {"inp": blob}


def kernel(embeddings, W1, b1, W2, b2):
    plan = _make_plan(np.asarray(W2).reshape(H))
    perm, sgn = plan[0], plan[1]
    nc = _get_nc(plan)
    in_maps = [
        _prep_core(core, embeddings, W1, b1, W2, b2, perm, sgn)
        for core in range(NCORES)
    ]
    res = run_bass_kernel_spmd(nc, in_maps, list(range(NCORES)))
    out = np.empty((B, N, N), np.float32)
    for core in range(NCORES):
        b = core // 2
        i0 = (core % 2) * ROWS
        out[b, i0 : i0 + ROWS, :] = res.results[core]["out"]
    return out
